# revision 22
# baseline (speedup 1.0000x reference)
"""Local (windowed) attention with RoPE for Trainium2, SPMD over 8 NeuronCores.

Reference semantics (nn_LocalAttention): B,H,N,D = 4,16,4096,64, window=128,
look_backward=1, look_forward=0, pad_value=-1 (pad applies to k/v VALUES and
to the position ids; padded keys end up unmasked all -1.0 vectors).

Sharding: merged (B*H)=64 leading dim split across 8 cores, 8 slices each.
Everything else runs per-core with no collectives.
"""

import numpy as np
import ml_dtypes

import concourse.bass as bass
import concourse.bacc as bacc
import concourse.mybir as mybir
import concourse.tile as tile
from concourse.bass_utils import run_bass_kernel_spmd

F32 = mybir.dt.float32
BF16 = mybir.dt.bfloat16
NP_BF16 = ml_dtypes.bfloat16

B, H, N, D = 4, 16, 4096, 64
W = 128                    # window size
NCORES = 8
BH = B * H
BH_PER_CORE = BH // NCORES
SCALE = float(D) ** -0.5
HD = D // 2


def rope_tables(n):
    """cos/sin tables matching the reference's fp32 computation.

    sinm folds the rotate_half sign: q'[d] = q[d]*cos[d] + q[(d+32)%64]*sinm[d].
    """
    inv_freq = 1.0 / (10000.0 ** (np.arange(0, D, 2, dtype=np.float32) / np.float32(D)))
    t = np.arange(n, dtype=np.float32)
    half = t[:, None] * inv_freq[None, :]
    freqs = np.concatenate([half, half], axis=-1)  # [n, D]
    cos = np.cos(freqs).astype(np.float32)
    sin = np.sin(freqs).astype(np.float32)
    sinm = np.concatenate([-sin[:, :HD], sin[:, HD:]], axis=-1)
    return cos, sinm


def host_consts(n):
    cos, sinm = rope_tables(n)
    # tri[j, i] = 1 where key j <= query i (window-local causal keep-mask)
    j = np.arange(W)[:, None]
    i = np.arange(W)[None, :]
    tri = (j <= i).astype(NP_BF16)
    ident = np.eye(D + 1, dtype=np.float32)
    return {
        "cos_t": cos.astype(NP_BF16),
        "sinm_t": sinm.astype(NP_BF16),
        "tri": tri,
        "id65": ident,
    }


def build_nc(bh_per_core=BH_PER_CORE, n=N):
    nw = n // W
    assert nw % 2 == 0
    ns = nw // 2  # transpose slabs (2 windows each)

    nc = bacc.Bacc(None, target_bir_lowering=False)
    # q,k,v stay separate DRAM tensors: one fused 96MB put measured equal
    # to three 32MB puts on the relay, and separate tensors let the host
    # bf16 cast of k/v hide under q's in-flight transfer.
    q_d = nc.dram_tensor("q", [bh_per_core, n, D], BF16, kind="ExternalInput")
    k_d = nc.dram_tensor("k", [bh_per_core, n, D], BF16, kind="ExternalInput")
    v_d = nc.dram_tensor("v", [bh_per_core, n, D], BF16, kind="ExternalInput")
    cos_d = nc.dram_tensor("cos_t", [n, D], BF16, kind="ExternalInput")
    sinm_d = nc.dram_tensor("sinm_t", [n, D], BF16, kind="ExternalInput")
    tri_d = nc.dram_tensor("tri", [W, W], BF16, kind="ExternalInput")
    id_d = nc.dram_tensor("id65", [D + 1, D + 1], F32, kind="ExternalInput")
    o_d = nc.dram_tensor("out", [bh_per_core, n, D], BF16, kind="ExternalOutput")

    def nat(ap):  # DRAM [n, D] -> [t, w, d] token-in-window on partitions
        return ap.rearrange("(w t) d -> t w d", t=W)

    with tile.TileContext(nc) as tc:
        with (
            tc.tile_pool(name="const", bufs=1) as constp,
            tc.tile_pool(name="io", bufs=2) as iop,
            tc.tile_pool(name="rope", bufs=2) as ropep,
            tc.tile_pool(name="stk", bufs=2) as stkp,
            tc.tile_pool(name="esb", bufs=4) as ep,
            tc.tile_pool(name="otsb", bufs=6) as otp,
            tc.tile_pool(name="rsb", bufs=3) as rp,
            tc.tile_pool(name="stage", bufs=2) as stagep,
            tc.tile_pool(name="psim", bufs=2, space="PSUM") as psimp,
            tc.tile_pool(name="pS", bufs=4, space="PSUM") as pSp,
            tc.tile_pool(name="pO", bufs=2, space="PSUM") as pOp,
        ):
            cos_sb = constp.tile([W, nw, D], BF16, tag="cos")
            nc.sync.dma_start(out=cos_sb, in_=nat(cos_d))
            sinm_sb = constp.tile([W, nw, D], BF16, tag="sinm")
            nc.sync.dma_start(out=sinm_sb, in_=nat(sinm_d))
            tri_sb = constp.tile([W, W], BF16, tag="tri")
            nc.sync.dma_start(out=tri_sb, in_=tri_d[:])
            id_sb = constp.tile([D + 1, D + 1], F32, tag="id65")
            nc.sync.dma_start(out=id_sb, in_=id_d[:])
            kpadT = constp.tile([D, W], BF16, tag="kpadT")
            nc.vector.memset(kpadT[:], -1.0)
            vpad = constp.tile([W, D + 1], BF16, tag="vpad")
            nc.vector.memset(vpad[:], -1.0)
            nc.vector.memset(vpad[:, D : D + 1], 1.0)

            for bh in range(bh_per_core):
                qn = iop.tile([W, nw, D], BF16, tag="qn")
                nc.sync.dma_start(out=qn[:], in_=nat(q_d[bh]))
                kn = iop.tile([W, nw, D], BF16, tag="kn")
                nc.sync.dma_start(out=kn[:], in_=nat(k_d[bh]))
                # v lands directly in its ones-column layout (denominator row)
                vb = ropep.tile([W, nw, D + 1], BF16, tag="vb")
                if bh < 2:  # ones column persists per pool slot
                    nc.vector.memset(vb[:, :, D : D + 1], 1.0)
                nc.sync.dma_start(out=vb[:, :, 0:D], in_=nat(v_d[bh]))

                # ---- RoPE (bf16, natural layout) ----
                # Output tiles are [W, nw, 2D] with d-columns D:2D zero -- the
                # XBAR transpose then puts every window's d-major tile at
                # partitions 0:64 (uniform matmul base partition).
                def rope(xb, tag):
                    xr = ropep.tile([W, nw, D], BF16, tag=tag + "r")
                    nc.vector.tensor_mul(
                        out=xr[:, :, 0:HD], in0=xb[:, :, HD:D], in1=sinm_sb[:, :, 0:HD]
                    )
                    nc.vector.tensor_mul(
                        out=xr[:, :, HD:D], in0=xb[:, :, 0:HD], in1=sinm_sb[:, :, HD:D]
                    )
                    xp = ropep.tile([W, nw, 2 * D], BF16, tag=tag + "p")
                    if bh < 2:  # zero the pad lanes once per pool slot
                        nc.vector.memset(xp[:, :, D : 2 * D], 0.0)
                    nc.vector.tensor_mul(out=xp[:, :, 0:D], in0=xb[:], in1=cos_sb[:])
                    nc.vector.tensor_add(
                        out=xp[:, :, 0:D], in0=xp[:, :, 0:D], in1=xr[:]
                    )
                    return xp

                qp = rope(qn, "q")
                kp = rope(kn, "k")

                # ---- d-major via XBAR dma transpose ----
                # stq[p, w, t]: p<64 -> d of window w; p>=64 -> zero pad
                stq = stkp.tile([W, nw, W], BF16, tag="stq")
                nc.sync.dma_start(
                    out=stq[:], in_=qp.rearrange("t w d -> t (w d)"), transpose=True
                )
                stk = stkp.tile([W, nw, W], BF16, tag="stk")
                nc.sync.dma_start(
                    out=stk[:], in_=kp.rearrange("t w d -> t (w d)"), transpose=True
                )

                def qT(w):  # [64, 128] moving operand for queries of window w
                    return stq[0:D, w, :]

                def kT(w):  # [64, 128] stationary operand for keys of window w
                    return stk[0:D, w, :]

                # groups of key blocks: g=0 -> (pad, 0); 1..ns-1 -> (2g-1, 2g);
                # g=ns -> (nw-1,)
                e_tiles = {}  # c -> (E tile, slot)
                o_quads = {}
                stage_sb = stagep.tile([W, nw, D], BF16, tag="stage")

                def do_window(w):
                    # out^T (and denom) for window w: accumulate both key
                    # blocks' PV into one PSUM tile, evacuate, transpose.
                    et0, sl0 = e_tiles[w - 1]
                    et1, sl1 = e_tiles[w]
                    pw = pSp.tile([D + 1, W], F32, tag="s", name="pw")
                    if w == 0:
                        nc.tensor.matmul(
                            pw[:], vpad[:], et0[:, sl0, 0:W], start=True, stop=False
                        )
                    else:
                        nc.tensor.matmul(
                            pw[:], vb[:, w - 1, :], et0[:, sl0, W : 2 * W],
                            start=True, stop=False,
                        )
                    nc.tensor.matmul(
                        pw[:], vb[:, w, :], et1[:, sl1, 0:W], start=False, stop=True
                    )
                    ot = otp.tile([D + 1, W], F32, tag="ot")
                    if w % 4 == 2:  # shed some PSUM-evac load from DVE to ACT
                        nc.scalar.copy(out=ot[:], in_=pw[:])
                    else:
                        nc.vector.tensor_copy(out=ot[:], in_=pw[:])
                    qi = w // 4
                    if qi not in o_quads:
                        o_quads[qi] = pOp.tile([W, 4, D + 1], F32, tag="oq", name="oq")
                    oq = o_quads[qi]
                    sl = w % 4
                    nc.tensor.transpose(oq[:, sl, :], ot[:], id_sb[:])
                    if sl == 3 or w == nw - 1:
                        nsl = sl + 1
                        r = rp.tile([W, 4], F32, tag="r")
                        nc.vector.reciprocal(
                            out=r[:, 0:nsl], in_=oq[:, 0:nsl, D : D + 1]
                        )
                        for j in range(nsl):
                            ww = qi * 4 + j
                            nc.scalar.activation(
                                out=stage_sb[:, ww, :],
                                in_=oq[:, j, 0:D],
                                func=mybir.ActivationFunctionType.Copy,
                                scale=r[:, j : j + 1],
                            )

                for g in range(ns + 1):
                    blocks = (
                        [-1, 0] if g == 0 else ([nw - 1] if g == ns else [2 * g - 1, 2 * g])
                    )
                    simt = psimp.tile([W, 2, 2 * W], F32, tag="sim")
                    et = ep.tile([W, 2, 2 * W], BF16, tag="e")
                    for sl, c in enumerate(blocks):
                        last = c == nw - 1
                        if c == -1:
                            nc.tensor.matmul(
                                simt[:, sl, 0:W], kpadT[:], qT(0), start=True, stop=True
                            )
                        else:
                            nc.tensor.matmul(
                                simt[:, sl, 0:W], kT(c), qT(c), start=True, stop=True
                            )
                            if not last:
                                nc.tensor.matmul(
                                    simt[:, sl, W : 2 * W],
                                    kT(c),
                                    qT(c + 1),
                                    start=True,
                                    stop=True,
                                )
                    # exp (scale folded); masked entries fixed up after
                    if g == 0:
                        nc.scalar.activation(
                            out=et[:, 0, 0:W], in_=simt[:, 0, 0:W],
                            func=mybir.ActivationFunctionType.Exp, scale=SCALE,
                        )
                        nc.scalar.activation(
                            out=et[:, 1, :], in_=simt[:, 1, :],
                            func=mybir.ActivationFunctionType.Exp, scale=SCALE,
                        )
                        nc.vector.tensor_mul(
                            out=et[:, 1, 0:W], in0=et[:, 1, 0:W], in1=tri_sb[:]
                        )
                    elif g == ns:
                        nc.scalar.activation(
                            out=et[:, 0, 0:W], in_=simt[:, 0, 0:W],
                            func=mybir.ActivationFunctionType.Exp, scale=SCALE,
                        )
                        nc.vector.tensor_mul(
                            out=et[:, 0, 0:W], in0=et[:, 0, 0:W], in1=tri_sb[:]
                        )
                    else:
                        nc.scalar.activation(
                            out=et[:, :, :], in_=simt[:, :, :],
                            func=mybir.ActivationFunctionType.Exp, scale=SCALE,
                        )
                        for sl in range(2):
                            nc.vector.tensor_mul(
                                out=et[:, sl, 0:W], in0=et[:, sl, 0:W], in1=tri_sb[:]
                            )
                    for sl, c in enumerate(blocks):
                        e_tiles[c] = (et, sl)
                    # windows ready after this group
                    for w in ([0] if g == 0 else ([nw - 1] if g == ns else [2 * g - 1, 2 * g])):
                        do_window(w)
                        e_tiles.pop(w - 1, None)

                nc.sync.dma_start(out=nat(o_d[bh]), in_=stage_sb[:])

    nc.finalize()
    return nc


_built = {}
TRACE = False
LAST_RESULT = None


def _get_nc(bh_per_core=BH_PER_CORE, n=N):
    key = (bh_per_core, n)
    if key not in _built:
        _built[key] = build_nc(bh_per_core, n)
    return _built[key]


_runner = None
CHUNKS = 1  # pipeline chunks along the per-core bh dim (2 measured slower)
DONATE_ZEROS = False  # kernel writes every output element; skip zero-donation


def _make_runner(chunks=CHUNKS):
    """Build the jitted SPMD executable ONCE and reuse it across calls.

    run_bass_kernel_spmd constructs a fresh jax.jit(shard_map(...)) closure
    per invocation, so every warm call re-traces + re-lowers + re-runs
    neuronxcc. Caching the jitted callable turns warm calls into pure
    dispatch + transfer + execute.

    With chunks>1 the per-core bh loop is split into `chunks` sequential
    device launches so chunk j's execute hides under chunk j+1's H2D.
    All D2H happens after all H2D: the axon relay serializes transfers
    and concurrent bidirectional traffic slows both directions down.
    """
    import jax
    import jax.numpy as jnp
    from jax.experimental.shard_map import shard_map
    from jax.sharding import Mesh, NamedSharding, PartitionSpec

    from concourse.bass2jax import (
        _bass_exec_p,
        install_neuronx_cc_hook,
        partition_id_tensor,
    )

    install_neuronx_cc_hook()
    assert BH_PER_CORE % chunks == 0
    bh_chunk = BH_PER_CORE // chunks
    nc = _get_nc(bh_chunk)
    assert not (nc.dbg_addr is not None and nc.dbg_callbacks)
    partition_name = nc.partition_id_tensor.name if nc.partition_id_tensor else None

    in_names = []
    out_names = []
    out_avals = []
    zero_shapes = []
    for alloc in nc.m.functions[0].allocations:
        if not isinstance(alloc, mybir.MemoryLocationSet):
            continue
        name = alloc.memorylocations[0].name
        if alloc.kind == "ExternalInput":
            if name != partition_name:
                in_names.append(name)
        elif alloc.kind == "ExternalOutput":
            out_names.append(name)
            shape = tuple(alloc.tensor_shape)
            dtype = mybir.dt.np(alloc.dtype)
            out_avals.append(jax.core.ShapedArray(shape, dtype))
            zero_shapes.append((shape, dtype))
    n_params = len(in_names)
    all_in_names = list(in_names) + (out_names if DONATE_ZEROS else [])
    if partition_name is not None:
        all_in_names.append(partition_name)

    def _body(*args):
        operands = list(args)
        if partition_name is not None:
            operands.append(partition_id_tensor())
        outs = _bass_exec_p.bind(
            *operands,
            out_avals=tuple(out_avals),
            in_names=tuple(all_in_names),
            out_names=tuple(out_names),
            lowering_input_output_aliases=(),
            sim_require_finite=True,
            sim_require_nnan=True,
            nc=nc,
        )
        return tuple(outs)

    devices = jax.devices()[:NCORES]
    assert len(devices) == NCORES
    mesh = Mesh(np.asarray(devices), ("core",))
    nspec = n_params + (len(out_names) if DONATE_ZEROS else 0)
    sharded = jax.jit(
        shard_map(
            _body,
            mesh=mesh,
            in_specs=(PartitionSpec("core"),) * nspec,
            out_specs=(PartitionSpec("core"),) * len(out_names),
            check_rep=False,
        ),
        donate_argnums=tuple(range(n_params, nspec)),
        keep_unused=True,
    )

    out_sharding = NamedSharding(mesh, PartitionSpec("core"))
    zeros_fns = (
        [
            jax.jit(
                (lambda sh, dt: (lambda: jnp.zeros((NCORES * sh[0], *sh[1:]), dt)))(
                    sh, dt
                ),
                out_shardings=out_sharding,
            )
            for sh, dt in zero_shapes
        ]
        if DONATE_ZEROS
        else []
    )

    # global (concat-over-cores) constant operands: device_put ONCE so warm
    # calls don't re-transfer them
    consts = host_consts(N)
    if nc.dbg_addr is not None:
        consts[nc.dbg_addr.name] = np.zeros((1, 2), np.uint32)
    const_global = {
        name: jax.device_put(
            np.ascontiguousarray(np.tile(arr, (NCORES,) + (1,) * (arr.ndim - 1))),
            out_sharding,
        )
        for name, arr in consts.items()
    }

    def run(q, k, v):
        # interleave host bf16 casts with async H2D so the k/v casts hide
        # under q's in-flight transfer; fetch outputs only after all H2D
        # is enqueued (the relay punishes concurrent bidirectional traffic)
        views = [
            np.asarray(x).reshape(NCORES, chunks, bh_chunk, N, D) for x in (q, k, v)
        ]
        dev = []
        for j in range(chunks):
            dev.append(
                tuple(
                    jax.device_put(
                        x[:, j].astype(NP_BF16).reshape(NCORES * bh_chunk, N, D),
                        out_sharding,
                    )
                    for x in views
                )
            )
        chunk_outs = []
        for j in range(chunks):
            per_name = {"q": dev[j][0], "k": dev[j][1], "v": dev[j][2], **const_global}
            args = [per_name[name] for name in in_names]
            zeros = [zf() for zf in zeros_fns]
            outs = sharded(*args, *zeros)
            chunk_outs.append({name: outs[i] for i, name in enumerate(out_names)})
        out = np.empty((NCORES, chunks, bh_chunk, N, D), np.float32)
        for j in range(chunks):
            o = np.asarray(chunk_outs[j]["out"])  # [NCORES*bh_chunk, N, D] bf16
            out[:, j] = o.reshape(NCORES, bh_chunk, N, D)
        return out.reshape(B, H, N, D)

    return run


def kernel(q, k, v):
    assert q.shape == (B, H, N, D)
    global _runner
    if _runner is None:
        _runner = _make_runner()
    return _runner(q, k, v)



# revision 24
# speedup vs baseline: 1.1707x; 1.1707x over previous
"""Local (windowed) attention with RoPE for Trainium2, SPMD over 8 NeuronCores.

Reference semantics (nn_LocalAttention): B,H,N,D = 4,16,4096,64, window=128,
look_backward=1, look_forward=0, pad_value=-1 (pad applies to k/v VALUES and
to the position ids; padded keys end up unmasked all -1.0 vectors).

Sharding: merged (B*H)=64 leading dim split across 8 cores, 8 slices each.
Everything else runs per-core with no collectives.

Wall-time design (the graded number is warm per-call wall time; the axon
relay moves bytes at ~75MB/s so host<->device transfer dominates):
- the jax.jit(shard_map(bass_exec)) executable is built once and cached
  (run_bass_kernel_spmd would rebuild + re-trace + re-run neuronxcc per
  call);
- q/k/v travel as bf16 (halves H2D to 96MB) and the output returns as
  bf16 (halves D2H to 32MB), fp8 fails the 2e-2 gate (measured 0.03-0.07);
- RoPE/mask/identity constants are device_put once;
- host bf16 casts interleave with async puts; no donated zero output
  buffers (the kernel writes every output element).
"""

import numpy as np
import ml_dtypes

import concourse.bass as bass
import concourse.bacc as bacc
import concourse.mybir as mybir
import concourse.tile as tile
from concourse.bass_utils import run_bass_kernel_spmd

F32 = mybir.dt.float32
BF16 = mybir.dt.bfloat16
NP_BF16 = ml_dtypes.bfloat16

B, H, N, D = 4, 16, 4096, 64
W = 128                    # window size
NCORES = 8
BH = B * H
BH_PER_CORE = BH // NCORES
SCALE = float(D) ** -0.5
HD = D // 2


def rope_tables(n):
    """cos/sin tables matching the reference's fp32 computation.

    sinm folds the rotate_half sign: q'[d] = q[d]*cos[d] + q[(d+32)%64]*sinm[d].
    """
    inv_freq = 1.0 / (10000.0 ** (np.arange(0, D, 2, dtype=np.float32) / np.float32(D)))
    t = np.arange(n, dtype=np.float32)
    half = t[:, None] * inv_freq[None, :]
    freqs = np.concatenate([half, half], axis=-1)  # [n, D]
    cos = np.cos(freqs).astype(np.float32)
    sin = np.sin(freqs).astype(np.float32)
    sinm = np.concatenate([-sin[:, :HD], sin[:, HD:]], axis=-1)
    return cos, sinm


def host_consts(n):
    cos, sinm = rope_tables(n)
    # tri[j, i] = 1 where key j <= query i (window-local causal keep-mask)
    j = np.arange(W)[:, None]
    i = np.arange(W)[None, :]
    tri = (j <= i).astype(NP_BF16)
    ident = np.eye(D + 1, dtype=np.float32)
    return {
        "cos_t": cos.astype(NP_BF16),
        "sinm_t": sinm.astype(NP_BF16),
        "tri": tri,
        "id65": ident,
    }


def build_nc(bh_per_core=BH_PER_CORE, n=N):
    nw = n // W
    assert nw % 2 == 0
    ns = nw // 2  # transpose slabs (2 windows each)

    nc = bacc.Bacc(None, target_bir_lowering=False)
    # q,k,v stay separate DRAM tensors: one fused 96MB put measured equal
    # to three 32MB puts on the relay, and separate tensors let the host
    # bf16 cast of k/v hide under q's in-flight transfer.
    q_d = nc.dram_tensor("q", [bh_per_core, n, D], BF16, kind="ExternalInput")
    k_d = nc.dram_tensor("k", [bh_per_core, n, D], BF16, kind="ExternalInput")
    v_d = nc.dram_tensor("v", [bh_per_core, n, D], BF16, kind="ExternalInput")
    cos_d = nc.dram_tensor("cos_t", [n, D], BF16, kind="ExternalInput")
    sinm_d = nc.dram_tensor("sinm_t", [n, D], BF16, kind="ExternalInput")
    tri_d = nc.dram_tensor("tri", [W, W], BF16, kind="ExternalInput")
    id_d = nc.dram_tensor("id65", [D + 1, D + 1], F32, kind="ExternalInput")
    o_d = nc.dram_tensor("out", [bh_per_core, n, D], BF16, kind="ExternalOutput")

    def nat(ap):  # DRAM [n, D] -> [t, w, d] token-in-window on partitions
        return ap.rearrange("(w t) d -> t w d", t=W)

    with tile.TileContext(nc) as tc:
        with (
            tc.tile_pool(name="const", bufs=1) as constp,
            tc.tile_pool(name="io", bufs=2) as iop,
            tc.tile_pool(name="rope", bufs=2) as ropep,
            tc.tile_pool(name="stk", bufs=2) as stkp,
            tc.tile_pool(name="esb", bufs=4) as ep,
            tc.tile_pool(name="otsb", bufs=6) as otp,
            tc.tile_pool(name="rsb", bufs=3) as rp,
            tc.tile_pool(name="stage", bufs=2) as stagep,
            tc.tile_pool(name="psim", bufs=2, space="PSUM") as psimp,
            tc.tile_pool(name="pS", bufs=4, space="PSUM") as pSp,
            tc.tile_pool(name="pO", bufs=2, space="PSUM") as pOp,
        ):
            cos_sb = constp.tile([W, nw, D], BF16, tag="cos")
            nc.sync.dma_start(out=cos_sb, in_=nat(cos_d))
            sinm_sb = constp.tile([W, nw, D], BF16, tag="sinm")
            nc.sync.dma_start(out=sinm_sb, in_=nat(sinm_d))
            tri_sb = constp.tile([W, W], BF16, tag="tri")
            nc.sync.dma_start(out=tri_sb, in_=tri_d[:])
            id_sb = constp.tile([D + 1, D + 1], F32, tag="id65")
            nc.sync.dma_start(out=id_sb, in_=id_d[:])
            kpadT = constp.tile([D, W], BF16, tag="kpadT")
            nc.vector.memset(kpadT[:], -1.0)
            vpad = constp.tile([W, D + 1], BF16, tag="vpad")
            nc.vector.memset(vpad[:], -1.0)
            nc.vector.memset(vpad[:, D : D + 1], 1.0)

            for bh in range(bh_per_core):
                qn = iop.tile([W, nw, D], BF16, tag="qn")
                nc.sync.dma_start(out=qn[:], in_=nat(q_d[bh]))
                kn = iop.tile([W, nw, D], BF16, tag="kn")
                nc.sync.dma_start(out=kn[:], in_=nat(k_d[bh]))
                # v lands directly in its ones-column layout (denominator row)
                vb = ropep.tile([W, nw, D + 1], BF16, tag="vb")
                if bh < 2:  # ones column persists per pool slot
                    nc.vector.memset(vb[:, :, D : D + 1], 1.0)
                nc.sync.dma_start(out=vb[:, :, 0:D], in_=nat(v_d[bh]))

                # ---- RoPE (bf16, natural layout) ----
                # Output tiles are [W, nw, 2D] with d-columns D:2D zero -- the
                # XBAR transpose then puts every window's d-major tile at
                # partitions 0:64 (uniform matmul base partition).
                def rope(xb, tag):
                    xr = ropep.tile([W, nw, D], BF16, tag=tag + "r")
                    nc.vector.tensor_mul(
                        out=xr[:, :, 0:HD], in0=xb[:, :, HD:D], in1=sinm_sb[:, :, 0:HD]
                    )
                    nc.vector.tensor_mul(
                        out=xr[:, :, HD:D], in0=xb[:, :, 0:HD], in1=sinm_sb[:, :, HD:D]
                    )
                    xp = ropep.tile([W, nw, 2 * D], BF16, tag=tag + "p")
                    if bh < 2:  # zero the pad lanes once per pool slot
                        nc.vector.memset(xp[:, :, D : 2 * D], 0.0)
                    nc.vector.tensor_mul(out=xp[:, :, 0:D], in0=xb[:], in1=cos_sb[:])
                    nc.vector.tensor_add(
                        out=xp[:, :, 0:D], in0=xp[:, :, 0:D], in1=xr[:]
                    )
                    return xp

                qp = rope(qn, "q")
                kp = rope(kn, "k")

                # ---- d-major via XBAR dma transpose ----
                # stq[p, w, t]: p<64 -> d of window w; p>=64 -> zero pad
                stq = stkp.tile([W, nw, W], BF16, tag="stq")
                nc.sync.dma_start(
                    out=stq[:], in_=qp.rearrange("t w d -> t (w d)"), transpose=True
                )
                stk = stkp.tile([W, nw, W], BF16, tag="stk")
                nc.sync.dma_start(
                    out=stk[:], in_=kp.rearrange("t w d -> t (w d)"), transpose=True
                )

                def qT(w):  # [64, 128] moving operand for queries of window w
                    return stq[0:D, w, :]

                def kT(w):  # [64, 128] stationary operand for keys of window w
                    return stk[0:D, w, :]

                # groups of key blocks: g=0 -> (pad, 0); 1..ns-1 -> (2g-1, 2g);
                # g=ns -> (nw-1,)
                e_tiles = {}  # c -> (E tile, slot)
                o_quads = {}
                stage_sb = stagep.tile([W, nw, D], BF16, tag="stage")

                def do_window(w):
                    # out^T (and denom) for window w: accumulate both key
                    # blocks' PV into one PSUM tile, evacuate, transpose.
                    et0, sl0 = e_tiles[w - 1]
                    et1, sl1 = e_tiles[w]
                    pw = pSp.tile([D + 1, W], F32, tag="s", name="pw")
                    if w == 0:
                        nc.tensor.matmul(
                            pw[:], vpad[:], et0[:, sl0, 0:W], start=True, stop=False
                        )
                    else:
                        nc.tensor.matmul(
                            pw[:], vb[:, w - 1, :], et0[:, sl0, W : 2 * W],
                            start=True, stop=False,
                        )
                    nc.tensor.matmul(
                        pw[:], vb[:, w, :], et1[:, sl1, 0:W], start=False, stop=True
                    )
                    ot = otp.tile([D + 1, W], F32, tag="ot")
                    if w % 4 == 2:  # shed some PSUM-evac load from DVE to ACT
                        nc.scalar.copy(out=ot[:], in_=pw[:])
                    else:
                        nc.vector.tensor_copy(out=ot[:], in_=pw[:])
                    qi = w // 4
                    if qi not in o_quads:
                        o_quads[qi] = pOp.tile([W, 4, D + 1], F32, tag="oq", name="oq")
                    oq = o_quads[qi]
                    sl = w % 4
                    nc.tensor.transpose(oq[:, sl, :], ot[:], id_sb[:])
                    if sl == 3 or w == nw - 1:
                        nsl = sl + 1
                        r = rp.tile([W, 4], F32, tag="r")
                        nc.vector.reciprocal(
                            out=r[:, 0:nsl], in_=oq[:, 0:nsl, D : D + 1]
                        )
                        for j in range(nsl):
                            ww = qi * 4 + j
                            nc.scalar.activation(
                                out=stage_sb[:, ww, :],
                                in_=oq[:, j, 0:D],
                                func=mybir.ActivationFunctionType.Copy,
                                scale=r[:, j : j + 1],
                            )

                for g in range(ns + 1):
                    blocks = (
                        [-1, 0] if g == 0 else ([nw - 1] if g == ns else [2 * g - 1, 2 * g])
                    )
                    simt = psimp.tile([W, 2, 2 * W], F32, tag="sim")
                    et = ep.tile([W, 2, 2 * W], BF16, tag="e")
                    for sl, c in enumerate(blocks):
                        last = c == nw - 1
                        if c == -1:
                            nc.tensor.matmul(
                                simt[:, sl, 0:W], kpadT[:], qT(0), start=True, stop=True
                            )
                        else:
                            nc.tensor.matmul(
                                simt[:, sl, 0:W], kT(c), qT(c), start=True, stop=True
                            )
                            if not last:
                                nc.tensor.matmul(
                                    simt[:, sl, W : 2 * W],
                                    kT(c),
                                    qT(c + 1),
                                    start=True,
                                    stop=True,
                                )
                    # exp (scale folded); masked entries fixed up after
                    if g == 0:
                        nc.scalar.activation(
                            out=et[:, 0, 0:W], in_=simt[:, 0, 0:W],
                            func=mybir.ActivationFunctionType.Exp, scale=SCALE,
                        )
                        nc.scalar.activation(
                            out=et[:, 1, :], in_=simt[:, 1, :],
                            func=mybir.ActivationFunctionType.Exp, scale=SCALE,
                        )
                        nc.vector.tensor_mul(
                            out=et[:, 1, 0:W], in0=et[:, 1, 0:W], in1=tri_sb[:]
                        )
                    elif g == ns:
                        nc.scalar.activation(
                            out=et[:, 0, 0:W], in_=simt[:, 0, 0:W],
                            func=mybir.ActivationFunctionType.Exp, scale=SCALE,
                        )
                        nc.vector.tensor_mul(
                            out=et[:, 0, 0:W], in0=et[:, 0, 0:W], in1=tri_sb[:]
                        )
                    else:
                        nc.scalar.activation(
                            out=et[:, :, :], in_=simt[:, :, :],
                            func=mybir.ActivationFunctionType.Exp, scale=SCALE,
                        )
                        for sl in range(2):
                            nc.vector.tensor_mul(
                                out=et[:, sl, 0:W], in0=et[:, sl, 0:W], in1=tri_sb[:]
                            )
                    for sl, c in enumerate(blocks):
                        e_tiles[c] = (et, sl)
                    # windows ready after this group
                    for w in ([0] if g == 0 else ([nw - 1] if g == ns else [2 * g - 1, 2 * g])):
                        do_window(w)
                        e_tiles.pop(w - 1, None)

                nc.sync.dma_start(out=nat(o_d[bh]), in_=stage_sb[:])

    nc.finalize()
    return nc


_built = {}
TRACE = False
LAST_RESULT = None


def _get_nc(bh_per_core=BH_PER_CORE, n=N):
    key = (bh_per_core, n)
    if key not in _built:
        _built[key] = build_nc(bh_per_core, n)
    return _built[key]


_runner = None
CHUNKS = 1  # pipeline chunks along the per-core bh dim (2 measured slower)
DONATE_ZEROS = False  # kernel writes every output element; skip zero-donation


def _make_runner(chunks=CHUNKS):
    """Build the jitted SPMD executable ONCE and reuse it across calls.

    run_bass_kernel_spmd constructs a fresh jax.jit(shard_map(...)) closure
    per invocation, so every warm call re-traces + re-lowers + re-runs
    neuronxcc. Caching the jitted callable turns warm calls into pure
    dispatch + transfer + execute.

    With chunks>1 the per-core bh loop is split into `chunks` sequential
    device launches so chunk j's execute hides under chunk j+1's H2D.
    All D2H happens after all H2D: the axon relay serializes transfers
    and concurrent bidirectional traffic slows both directions down.
    """
    import jax
    import jax.numpy as jnp
    from jax.experimental.shard_map import shard_map
    from jax.sharding import Mesh, NamedSharding, PartitionSpec

    from concourse.bass2jax import (
        _bass_exec_p,
        install_neuronx_cc_hook,
        partition_id_tensor,
    )

    install_neuronx_cc_hook()
    assert BH_PER_CORE % chunks == 0
    bh_chunk = BH_PER_CORE // chunks
    nc = _get_nc(bh_chunk)
    assert not (nc.dbg_addr is not None and nc.dbg_callbacks)
    partition_name = nc.partition_id_tensor.name if nc.partition_id_tensor else None

    in_names = []
    out_names = []
    out_avals = []
    zero_shapes = []
    for alloc in nc.m.functions[0].allocations:
        if not isinstance(alloc, mybir.MemoryLocationSet):
            continue
        name = alloc.memorylocations[0].name
        if alloc.kind == "ExternalInput":
            if name != partition_name:
                in_names.append(name)
        elif alloc.kind == "ExternalOutput":
            out_names.append(name)
            shape = tuple(alloc.tensor_shape)
            dtype = mybir.dt.np(alloc.dtype)
            out_avals.append(jax.core.ShapedArray(shape, dtype))
            zero_shapes.append((shape, dtype))
    n_params = len(in_names)
    all_in_names = list(in_names) + (out_names if DONATE_ZEROS else [])
    if partition_name is not None:
        all_in_names.append(partition_name)

    def _body(*args):
        operands = list(args)
        if partition_name is not None:
            operands.append(partition_id_tensor())
        outs = _bass_exec_p.bind(
            *operands,
            out_avals=tuple(out_avals),
            in_names=tuple(all_in_names),
            out_names=tuple(out_names),
            lowering_input_output_aliases=(),
            sim_require_finite=True,
            sim_require_nnan=True,
            nc=nc,
        )
        return tuple(outs)

    devices = jax.devices()[:NCORES]
    assert len(devices) == NCORES
    mesh = Mesh(np.asarray(devices), ("core",))
    nspec = n_params + (len(out_names) if DONATE_ZEROS else 0)
    sharded = jax.jit(
        shard_map(
            _body,
            mesh=mesh,
            in_specs=(PartitionSpec("core"),) * nspec,
            out_specs=(PartitionSpec("core"),) * len(out_names),
            check_rep=False,
        ),
        donate_argnums=tuple(range(n_params, nspec)),
        keep_unused=True,
    )

    out_sharding = NamedSharding(mesh, PartitionSpec("core"))
    zeros_fns = (
        [
            jax.jit(
                (lambda sh, dt: (lambda: jnp.zeros((NCORES * sh[0], *sh[1:]), dt)))(
                    sh, dt
                ),
                out_shardings=out_sharding,
            )
            for sh, dt in zero_shapes
        ]
        if DONATE_ZEROS
        else []
    )

    # global (concat-over-cores) constant operands: device_put ONCE so warm
    # calls don't re-transfer them
    consts = host_consts(N)
    if nc.dbg_addr is not None:
        consts[nc.dbg_addr.name] = np.zeros((1, 2), np.uint32)
    const_global = {
        name: jax.device_put(
            np.ascontiguousarray(np.tile(arr, (NCORES,) + (1,) * (arr.ndim - 1))),
            out_sharding,
        )
        for name, arr in consts.items()
    }

    def run(q, k, v):
        # interleave host bf16 casts with async H2D so the k/v casts hide
        # under q's in-flight transfer; fetch outputs only after all H2D
        # is enqueued (the relay punishes concurrent bidirectional traffic)
        views = [
            np.asarray(x).reshape(NCORES, chunks, bh_chunk, N, D) for x in (q, k, v)
        ]
        dev = []
        for j in range(chunks):
            dev.append(
                tuple(
                    jax.device_put(
                        x[:, j].astype(NP_BF16).reshape(NCORES * bh_chunk, N, D),
                        out_sharding,
                    )
                    for x in views
                )
            )
        chunk_outs = []
        for j in range(chunks):
            per_name = {"q": dev[j][0], "k": dev[j][1], "v": dev[j][2], **const_global}
            args = [per_name[name] for name in in_names]
            zeros = [zf() for zf in zeros_fns]
            outs = sharded(*args, *zeros)
            chunk_outs.append({name: outs[i] for i, name in enumerate(out_names)})
        # fetch output shards async and convert each to f32 while later
        # shards are still on the wire (hides the bf16->f32 upcast)
        out = np.empty((NCORES, chunks, bh_chunk, N, D), np.float32)
        fetches = []
        for j in range(chunks):
            shards = chunk_outs[j]["out"].addressable_shards
            for s in shards:
                s.data.copy_to_host_async()
            fetches.append(shards)
        for j in range(chunks):
            for s in fetches[j]:
                c = (s.index[0].start or 0) // bh_chunk
                out[c, j] = np.asarray(s.data)  # [bh_chunk, N, D] bf16 -> f32
        return out.reshape(B, H, N, D)

    return run


def kernel(q, k, v):
    assert q.shape == (B, H, N, D)
    global _runner
    if _runner is None:
        _runner = _make_runner()
    return _runner(q, k, v)

